# revision 41
# baseline (speedup 1.0000x reference)
"""SAN Bottleneck (pairwise self-attention) Trainium2 kernel, v2.

Sharding: 8 cores x 7 output rows (H=56), each core handles BOTH batches.
The rel=64-channel tensors (x1/x2/feat/h2) pack the two batches across the
128 partitions, halving the free-dim size of the windowed subtract, relu,
mm1 (one K=128 block-diagonal matmul computes both batches per column) and
the h2 relu-evacuation.

Per-core pipeline (batchnorms folded into per-channel scale/bias on host;
4 row-chunks, piece-wise PSUM streaming for mm1/mm2):
  bn1+relu (ACT) -> x1/x2/x3 1x1 convs (bf16 matmuls, batch-stacked M=128)
  feat = x1 - window(x2)  (DVE fp16 2x, one op per di)  -> relu (DVE 4x)
  mm1: block-diag cw1 (K=128) + position pass (K=2, rsubp streamed from HBM)
  relu evac (ACT) -> mm2 per batch (K=64, M=128 4x-replicated heads)
  exp evac (ACT, no bias: softmax is shift-invariant per (head,q))
  aggregation: windowed products + pairwise 49-tap tree (DVE fp16 2x)
  softmax normalizer: tree L1/L2 on Pool (GPSIMD), tail levels on DVE,
  reciprocal + scale (DVE), bn2+relu (ACT), wc conv (PE) -> plain evac;
  the final +bc bias and +x identity residual are added on the host during
  the gather (saves the DVE residual pass).

Channel permutation for x3/aggregation (s-split): partition
p <-> channel 8*(p//4)+4t+(p%4), so one 4x-replicated exp tensor serves
both 128-channel tiles.
"""

import numpy as np
import ml_dtypes

bf16_np = ml_dtypes.bfloat16

K = 7
PAD = 3
EPS = 1e-5
B, C, H, W = 2, 256, 56, 56
RB = 7               # rows per core
NQ = RB * W          # 392 per batch
ROWS = RB + 2 * PAD  # 13
WP = W + 2 * PAD     # 62
K2 = K * K
CHUNKS = [(0, 1), (1, 2), (3, 2), (5, 2)]  # (row0, nrows)
P1 = 1024
PIECE = 1024

_BUILD_CACHE = {}


def _perm_channels():
    perm = np.zeros(256, np.int64)
    for t in range(2):
        for p in range(128):
            perm[t * 128 + p] = 8 * (p // 4) + 4 * t + (p % 4)
    return perm


def _build_program():
    if "nc" in _BUILD_CACHE:
        return _BUILD_CACHE["nc"]
    import concourse.bass as bass
    import concourse.bacc as bacc
    import concourse.tile as tile
    import concourse.mybir as mybir
    from contextlib import ExitStack

    f32 = mybir.dt.float32
    f16 = mybir.dt.float16
    bf16 = mybir.dt.bfloat16
    Alu = mybir.AluOpType
    Act = mybir.ActivationFunctionType

    nc = bacc.Bacc("TRN2", target_bir_lowering=False, num_devices=8)

    xp_d = nc.dram_tensor("xp", [128, 4, ROWS * WP], f32, kind="ExternalInput")
    rsubp_d = nc.dram_tensor("rsubp", [2, K2 * NQ], f16, kind="ExternalInput")
    # packed weights: w1(2x64) | w2(2x64) | w3(2kt,2ot,128) | wc(...) |
    # cw1blk(128) | cw2(128)
    wpk_d = nc.dram_tensor("wpk", [128, 1280], bf16, kind="ExternalInput")
    cwf_d = nc.dram_tensor("cwf", [128, 256], f16, kind="ExternalInput")
    cw1pos_d = nc.dram_tensor("cw1pos", [2, 128], f16, kind="ExternalInput")
    scal_d = nc.dram_tensor("scal", [128, 13], f32, kind="ExternalInput")
    y_d = nc.dram_tensor("y", [4, 128, NQ], f32, kind="ExternalOutput")

    with tile.TileContext(nc) as tc, ExitStack() as stack:
        consts = stack.enter_context(tc.tile_pool(name="consts", bufs=1))
        xpp = stack.enter_context(tc.tile_pool(name="xpp", bufs=1))
        headsb = stack.enter_context(tc.tile_pool(name="headsb", bufs=1))
        featp = stack.enter_context(tc.tile_pool(name="featp", bufs=3))
        h2p = stack.enter_context(tc.tile_pool(name="h2p", bufs=3))
        e4p = stack.enter_context(tc.tile_pool(name="e4p", bufs=4))
        prodp = stack.enter_context(tc.tile_pool(name="prodp", bufs=3))
        zscp = stack.enter_context(tc.tile_pool(name="zscp", bufs=1))
        stripep = stack.enter_context(tc.tile_pool(name="stripep", bufs=2))
        smallp = stack.enter_context(tc.tile_pool(name="smallp", bufs=2))
        ps1p = stack.enter_context(tc.tile_pool(name="ps1p", bufs=1, space="PSUM"))
        ps2p = stack.enter_context(tc.tile_pool(name="ps2p", bufs=2, space="PSUM"))
        psxp = stack.enter_context(tc.tile_pool(name="psxp", bufs=1, space="PSUM"))

        scals = consts.tile([128, 13], f32, tag="scals")
        nc.sync.dma_start(out=scals[:], in_=scal_d[:])
        wpk = consts.tile([128, 1280], bf16, tag="wpk")
        cwf = consts.tile([128, 256], f16, tag="cwf")
        cwps = consts.tile([2, 128], f16, tag="cwps")
        w1s = wpk[:].rearrange("p (a b) -> p a b", b=64)[:, 0:2, :]
        w2s = wpk[:].rearrange("p (a b) -> p a b", b=64)[:, 2:4, :]
        w3s = wpk[:, 256:768].rearrange("p (kt ot m) -> p kt ot m",
                                        kt=2, ot=2)
        wcs = wpk[:, 768:1280].rearrange("p (kt ot m) -> p kt ot m",
                                         kt=2, ot=2)
        cw1s = cwf[:, 0:128]
        cw2s = cwf[:, 128:256]

        a1 = [scals[:, 0:1], scals[:, 1:2]]
        b1f = [scals[:, 2:3], scals[:, 3:4]]
        b1pd = scals[:, 4:5]
        b2pd = scals[:, 5:6]
        b2fd = scals[:, 6:7]
        a3p = [scals[:, 7:8], scals[:, 8:9]]
        b3fp = [scals[:, 9:10], scals[:, 10:11]]
        bcb = [scals[:, 11:12], scals[:, 12:13]]

        # input slab: slot (b*2+ct) of [128ch, ROWS*WP]; per-slot DMAs so the
        # first obn can start before the whole slab lands
        xps = xpp.tile([128, 4, ROWS * WP], f32, tag="xps")
        obn = headsb.tile([128, 4, ROWS * WP], bf16, tag="obn")
        for b in range(2):
            for ct in range(2):
                s = b * 2 + ct
                nc.sync.dma_start(out=xps[:, s, :], in_=xp_d[:, s, :])
                nc.scalar.activation(
                    out=obn[:, s, :], in_=xps[:, s, :],
                    func=Act.Relu, bias=b1f[ct], scale=a1[ct])
        nc.sync.dma_start(out=wpk[:], in_=wpk_d[:])
        nc.sync.dma_start(out=cwf[:], in_=cwf_d[:])
        nc.sync.dma_start(out=cwps[:], in_=cw1pos_d[:])

        def mm(out_ap, w_ap, rhs_ap, n, start, stop):
            # PE matmul: moving dim must be <= 512 per instruction
            for s in range(0, n, 512):
                sn = min(512, n - s)
                nc.tensor.matmul(out_ap[:, s:s + sn], w_ap,
                                 rhs_ap[:, s:s + sn], start=start, stop=stop)

        # x1s: [128 = 2b x 64rel, RB*W] f16 (center rows/cols only)
        x1s = headsb.tile([128, RB, W], f16, tag="x1s")
        psx1 = psxp.tile([128, PIECE], f32, tag="psx")
        for b in range(2):
            for kt in range(2):
                rhs = obn[:, b * 2 + kt, :].rearrange(
                    "p (r w) -> p r w", w=WP)[:, PAD:PAD + RB, PAD:PAD + W]
                nc.tensor.matmul(
                    psx1[64 * b:64 * b + 64, :NQ], w1s[:, kt, :], rhs,
                    start=(kt == 0), stop=(kt == 1))
        x1f = x1s[:].rearrange("p r w -> p (r w)")
        nc.scalar.activation(out=x1f[:, 0:W], in_=psx1[:, :W],
                             func=Act.Identity, bias=b1pd, scale=1.0)
        nc.scalar.activation(out=x1f[:, W:NQ], in_=psx1[:, W:NQ],
                             func=Act.Identity, bias=b1pd, scale=1.0)

        # x2s: [128 = 2b x 64rel, ROWS*WP] f16
        x2s = headsb.tile([128, ROWS * WP], f16, tag="x2s")
        psx2 = psxp.tile([128, PIECE], f32, tag="psx")
        for b in range(2):
            for kt in range(2):
                mm(psx2[64 * b:64 * b + 64, :], w2s[:, kt, :],
                   obn[:, b * 2 + kt, :], ROWS * WP, kt == 0, kt == 1)
        # evac rows 0..6 first so chunk 0's subtract can start early
        nc.scalar.activation(out=x2s[:, 0:7 * WP], in_=psx2[:, :7 * WP],
                             func=Act.Identity, bias=b2pd, scale=1.0)
        nc.scalar.activation(out=x2s[:, 7 * WP:], in_=psx2[:, 7 * WP:ROWS * WP],
                             func=Act.Identity, bias=b2pd, scale=1.0)

        # x3ps: slot (b*2+ot): [128 perm-ch, ROWS*WP] f16 (emitted after
        # phase1(0) so its evacs don't block the first h2/exp stream)
        x3ps = headsb.tile([128, 4, ROWS * WP], f16, tag="x3ps")

        def x3convs():
            for b in range(2):
                for ot in range(2):
                    ps3 = ps2p.tile([128, PIECE], f32, tag="ps2")
                    for kt in range(2):
                        mm(ps3, w3s[:, kt, ot, :],
                           obn[:, b * 2 + kt, :], ROWS * WP, kt == 0, kt == 1)
                    nc.scalar.activation(out=x3ps[:, b * 2 + ot, :],
                                         in_=ps3[:, :ROWS * WP], func=Act.Copy)

        chunk_state = {}
        # chunk-major offsets into rsubp_d's flat [K2*NQ]
        roffs = []
        acc = 0
        for (r0c, nr) in CHUNKS:
            roffs.append(acc)
            acc += K2 * nr * W

        def phase1(ci):
            (r0c, nr) = CHUNKS[ci]
            nqc = nr * W
            vc = K2 * nqc
            feat = featp.tile([128, vc], f16, tag="feat", name=f"feat{ci}")
            x1v = x1s[:, r0c:r0c + nr, :]
            # feat[k=(di,dj), q=(r,c)] = x1 - x2window ; one DVE op per di
            for di in range(K):
                x2w = bass.AP(
                    tensor=x2s.tensor, offset=x2s[:].offset + (r0c + di) * WP,
                    ap=[x2s[:].ap[0], [1, K], [WP, nr], [1, W]])
                x1w = bass.AP(
                    tensor=x1v.tensor, offset=x1v.offset,
                    ap=[x1v.ap[0], [0, K], x1v.ap[1], x1v.ap[2]])
                outw = bass.AP(
                    tensor=feat.tensor, offset=feat[:].offset + di * K * nqc,
                    ap=[feat[:].ap[0], [nqc, K], [W, nr], [1, W]])
                nc.vector.tensor_tensor(out=outw, in0=x1w, in1=x2w,
                                        op=Alu.subtract)
            nc.vector.tensor_scalar_max(out=feat[:], in0=feat[:],
                                        scalar1=0.0)

            h2 = h2p.tile([128, vc], f16, tag="h2", name=f"h2{ci}")
            for j0 in range(0, vc, P1):
                n = min(P1, vc - j0)
                stripe = stripep.tile([2, P1], f16, tag="stripe")
                nc.sync.dma_start(
                    out=stripe[:, :n],
                    in_=rsubp_d[:, roffs[ci] + j0:roffs[ci] + j0 + n])
                ps1 = ps1p.tile([128, P1], f32, tag="ps1")
                mm(ps1, cw1s[:], feat[:, j0:j0 + n], n, True, False)
                mm(ps1, cwps[:], stripe[:, :n], n, False, True)
                nc.scalar.activation(out=h2[:, j0:j0 + n], in_=ps1[:, :n],
                                     func=Act.Relu, bias=b2fd, scale=1.0)

            e4s = []
            for b in range(2):
                e4 = e4p.tile([128, vc], f16, tag="e4", name=f"e4_{ci}_{b}")
                e4s.append(e4)
                for j0 in range(0, vc, PIECE):
                    n = min(PIECE, vc - j0)
                    ps2 = ps2p.tile([128, PIECE], f32, tag="ps2")
                    mm(ps2, cw2s[64 * b:64 * b + 64, :],
                       h2[64 * b:64 * b + 64, j0:j0 + n], n, True, True)
                    nc.scalar.activation(out=e4[:, j0:j0 + n], in_=ps2[:, :n],
                                         func=Act.Exp)
            chunk_state[ci] = e4s

        def ksum_tree(eng, t, nqc):
            # in-place pairwise sum of 49 tap-planes [128, 49, nqc] -> [:,0,:]
            def v(k, n):
                return bass.AP(tensor=t.tensor, offset=t[:].offset + k * nqc,
                               ap=[t[:].ap[0], [nqc, n], [1, nqc]])
            for (a, b_, n) in [(0, 24, 24), (0, 12, 12), (0, 6, 6), (0, 3, 3)]:
                eng.tensor_tensor(out=v(a, n), in0=v(a, n), in1=v(b_, n),
                                  op=Alu.add)
            for b_ in (1, 2, 48):
                eng.tensor_tensor(out=v(0, 1), in0=v(0, 1), in1=v(b_, 1),
                                  op=Alu.add)

        def phase2(ci, b):
            (r0c, nr) = CHUNKS[ci]
            nqc = nr * W
            e4 = chunk_state[ci][b]

            # softmax normalizer: L1+L2 on Pool, small levels on DVE
            zsc = zscp.tile([128, 24, nqc], f16, tag="zsc", name=f"zsc{ci}{b}")
            def ev(k, n):
                return bass.AP(tensor=e4.tensor, offset=e4[:].offset + k * nqc,
                               ap=[e4[:].ap[0], [nqc, n], [1, nqc]])
            zeng = (nc.vector if (ci == len(CHUNKS) - 1 and b == 1)
                    else nc.gpsimd)
            zeng.tensor_tensor(out=zsc[:, 0:24, :], in0=ev(0, 24),
                               in1=ev(24, 24), op=Alu.add)
            zeng.tensor_tensor(out=zsc[:, 0:12, :], in0=zsc[:, 0:12, :],
                               in1=zsc[:, 12:24, :], op=Alu.add)
            for (a, b_, n) in [(0, 6, 6), (0, 3, 3)]:
                nc.vector.tensor_tensor(out=zsc[:, a:a + n, :],
                                        in0=zsc[:, a:a + n, :],
                                        in1=zsc[:, b_:b_ + n, :], op=Alu.add)
            for b_ in (1, 2):
                nc.vector.tensor_tensor(out=zsc[:, 0, :], in0=zsc[:, 0, :],
                                        in1=zsc[:, b_, :], op=Alu.add)
            nc.vector.tensor_tensor(out=zsc[:, 0, :], in0=zsc[:, 0, :],
                                    in1=ev(48, 1), op=Alu.add)

            zf = smallp.tile([128, nqc], f32, tag="zf")
            rz = smallp.tile([128, nqc], f32, tag="rz")
            nc.vector.tensor_copy(out=zf[:], in_=zsc[:, 0, :])
            nc.vector.reciprocal(out=rz[:], in_=zf[:])

            outb = []
            for t in range(2):
                prods = prodp.tile([128, K2, nqc], f16, tag="prods",
                                   name=f"prods{ci}{b}{t}")
                x3v = x3ps[:, b * 2 + t, :]
                for di in range(K):
                    x3w = bass.AP(
                        tensor=x3v.tensor,
                        offset=x3v.offset + (r0c + di) * WP,
                        ap=[x3v.ap[0], [1, K], [WP, nr], [1, W]])
                    e4w = bass.AP(
                        tensor=e4.tensor, offset=e4[:].offset + di * K * nqc,
                        ap=[e4[:].ap[0], [nqc, K], [W, nr], [1, W]])
                    outw = bass.AP(
                        tensor=prods.tensor,
                        offset=prods[:].offset + di * K * nqc,
                        ap=[prods[:].ap[0], [nqc, K], [W, nr], [1, W]])
                    nc.vector.tensor_tensor(out=outw, in0=e4w, in1=x3w,
                                            op=Alu.mult)
                ksum_tree(nc.vector, prods, nqc)

                ob = smallp.tile([128, nqc], f32, tag=f"ob{t}", name=f"ob{t}")
                ob2 = smallp.tile([128, nqc], bf16, tag=f"ob2{t}",
                                  name=f"ob2_{ci}{b}{t}")
                outb.append(ob2)
                nc.vector.tensor_tensor(out=ob[:], in0=prods[:, 0, :],
                                        in1=rz[:], op=Alu.mult)
                nc.scalar.activation(out=ob2[:], in_=ob[:], func=Act.Relu,
                                     bias=b3fp[t], scale=a3p[t])

            for oo in range(2):
                psw = psxp.tile([128, PIECE], f32, tag="psx")
                for kt in range(2):
                    mm(psw, wcs[:, kt, oo, :], outb[kt][:], nqc,
                       kt == 0, kt == 1)
                ysb = smallp.tile([128, nqc], f32, tag=f"ysb{oo}",
                                  name=f"ysb{ci}{b}{oo}")
                nc.scalar.activation(out=ysb[:], in_=psw[:, :nqc],
                                     func=Act.Copy)
                nc.sync.dma_start(
                    out=y_d[b * 2 + oo][:, r0c * W:(r0c + nr) * W],
                    in_=ysb[:])

        phase1(0)
        x3convs()
        phase1(1)
        phase2(0, 0)
        phase2(0, 1)
        phase1(2)
        phase2(1, 0)
        phase2(1, 1)
        phase1(3)
        phase2(2, 0)
        phase2(2, 1)
        phase2(3, 0)
        phase2(3, 1)

    nc.compile()
    _BUILD_CACHE["nc"] = nc
    return nc


def _host_prep(inputs):
    f = {k: np.asarray(v, np.float32) for k, v in inputs.items()}

    def fold(n):
        a = f[n + "_g"] / np.sqrt(f[n + "_rv"] + EPS)
        return a, f[n + "_b"] - f[n + "_rm"] * a

    a1, b1f = fold("bn1")
    ac, bc1 = fold("cwbn1")
    a2, b2f = fold("cwbn2")
    a3, b3f = fold("bn2")

    W1p = ac[:64, None] * f["w1"]
    b1p = ac[:64] * f["b1"] + bc1[:64]
    W2p = ac[:64, None] * f["w2"]
    b2p = ac[:64] * f["b2"]
    cw1p = a2[:, None] * f["cw1"]

    perm = _perm_channels()
    w3p = f["w3"][perm]
    a3p = a3[perm]
    b3fp = b3f[perm]
    rep = np.arange(128) // 4
    cw2r = f["cw2"][rep]

    # position encoding: relu(bn(subp)) on host
    locw = np.tile(np.linspace(-1.0, 1.0, W, dtype=np.float32)[None, :], (H, 1))
    loch = np.tile(np.linspace(-1.0, 1.0, H, dtype=np.float32)[:, None], (1, W))
    loc = np.stack([locw, loch], 0)
    p = np.einsum("chw,oc->ohw", loc, f["pw"]) + f["pb"][:, None, None]
    pp = np.pad(p, ((0, 0), (PAD, PAD), (PAD, PAD)), mode="reflect")
    pu = np.stack([pp[:, i:i + H, j:j + W] for i in range(K) for j in range(K)], 1)
    subp = p[:, None] - pu
    rsubp = np.maximum(ac[64:66, None, None, None] * subp
                       + bc1[64:66, None, None, None], 0).astype(np.float16)

    xpad = np.pad(f["x"], ((0, 0), (0, 0), (PAD, PAD), (PAD, PAD)),
                  mode="reflect")

    w1T = np.ascontiguousarray(W1p.T).reshape(2, 128, 64)
    w2T = np.ascontiguousarray(W2p.T).reshape(2, 128, 64)
    w3T = np.empty((2, 128, 2, 128), np.float32)
    wcT = np.empty((2, 128, 2, 128), np.float32)
    wc_perm = f["wc"][:, perm]
    for kt in range(2):
        for ot in range(2):
            w3T[kt, :, ot, :] = w3p[ot * 128:(ot + 1) * 128,
                                    kt * 128:(kt + 1) * 128].T
            wcT[kt, :, ot, :] = wc_perm[ot * 128:(ot + 1) * 128,
                                        kt * 128:(kt + 1) * 128].T

    cw1blk = np.zeros((128, 128), np.float32)
    cw1blk[0:64, 0:64] = cw1p[:, :64].T
    cw1blk[64:128, 64:128] = cw1p[:, :64].T
    cw1pos = np.zeros((2, 128), np.float32)
    cw1pos[:, 0:64] = cw1p[:, 64:66].T
    cw1pos[:, 64:128] = cw1p[:, 64:66].T
    cw2T = np.ascontiguousarray(np.concatenate([cw2r.T, cw2r.T], axis=0))

    scal = np.zeros((128, 13), np.float32)
    scal[:, 0] = a1[:128]; scal[:, 1] = a1[128:]
    scal[:, 2] = b1f[:128]; scal[:, 3] = b1f[128:]
    scal[:64, 4] = b1p; scal[64:, 4] = b1p
    scal[:64, 5] = b2p; scal[64:, 5] = b2p
    scal[:64, 6] = b2f; scal[64:, 6] = b2f
    scal[:, 7] = a3p[:128]; scal[:, 8] = a3p[128:]
    scal[:, 9] = b3fp[:128]; scal[:, 10] = b3fp[128:]
    scal[:, 11] = f["bc"][:128]; scal[:, 12] = f["bc"][128:]

    wpk = np.zeros((128, 1280), np.float32)
    wpk[:, 0:64] = w1T[0]; wpk[:, 64:128] = w1T[1]
    wpk[:, 128:192] = w2T[0]; wpk[:, 192:256] = w2T[1]
    wpk[:, 256:768] = w3T.reshape(2, 128, 256).transpose(1, 0, 2).reshape(
        128, 512)
    wpk[:, 768:1280] = wcT.reshape(2, 128, 256).transpose(1, 0, 2).reshape(
        128, 512)
    cwf = np.concatenate([cw1blk, cw2T], axis=1)
    shared = dict(wpk=wpk.astype(bf16_np), cwf=cwf.astype(np.float16),
                  cw1pos=cw1pos.astype(np.float16), scal=scal)

    in_maps = []
    for core in range(8):
        r0 = RB * core
        m = dict(shared)
        # xp: [128ch, 4 = (b*2+ct), ROWS*WP]
        slab = xpad[:, :, r0:r0 + ROWS, :]          # [2, 256, 13, 62]
        xp = np.empty((128, 4, ROWS * WP), np.float32)
        for b in range(2):
            for ct in range(2):
                xp[:, b * 2 + ct, :] = slab[b, ct * 128:(ct + 1) * 128].reshape(
                    128, ROWS * WP)
        m["xp"] = xp
        # rsubp: chunk-major flat [2, K2*NQ]
        rs = rsubp[:, :, r0:r0 + RB, :]             # [2, 49, 7, 56]
        parts = []
        for (r0c, nr) in CHUNKS:
            parts.append(rs[:, :, r0c:r0c + nr, :].reshape(2, -1))
        m["rsubp"] = np.ascontiguousarray(np.concatenate(parts, axis=1))
        in_maps.append(m)
    return in_maps


def kernel(**inputs):
    from concourse.bass_utils import run_bass_kernel_spmd
    nc = _build_program()
    in_maps = _host_prep(inputs)
    res = run_bass_kernel_spmd(nc, in_maps, core_ids=list(range(8)))
    global LAST_RESULTS
    LAST_RESULTS = res
    y = np.zeros((B, C, H, W), np.float32)
    for core in range(8):
        r0 = RB * core
        yc = res.results[core]["y"]                 # [4, 128, NQ]
        for b in range(2):
            for ot in range(2):
                y[b, ot * 128:(ot + 1) * 128, r0:r0 + RB, :] = (
                    yc[b * 2 + ot].reshape(128, RB, W))
    y += np.asarray(inputs["bc"], np.float32).reshape(1, C, 1, 1)
    y += np.asarray(inputs["x"], np.float32)
    return y


# revision 47
# speedup vs baseline: 1.0557x; 1.0557x over previous
"""SAN Bottleneck (pairwise self-attention) Trainium2 kernel, v2.

Sharding: 8 cores x 7 output rows (H=56), each core handles BOTH batches.
The rel=64-channel tensors (x1/x2/feat/h2) pack the two batches across the
128 partitions, halving the free-dim size of the windowed subtract, relu,
mm1 (one K=128 block-diagonal matmul computes both batches per column) and
the h2 relu-evacuation.

Per-core pipeline (batchnorms folded into per-channel scale/bias on host;
4 row-chunks, piece-wise PSUM streaming for mm1/mm2):
  bn1+relu (ACT) -> x1/x2/x3 1x1 convs (bf16 matmuls, batch-stacked M=128)
  feat = x1 - window(x2)  (DVE fp16 2x, one op per di)  -> relu (DVE 4x)
  mm1: block-diag cw1 (K=128) + position pass (K=2, rsubp streamed from HBM)
  relu evac (ACT) -> mm2 per batch (K=64, M=128 4x-replicated heads)
  exp evac (ACT, no bias: softmax is shift-invariant per (head,q))
  aggregation: windowed products + pairwise 49-tap tree (DVE fp16 2x)
  softmax normalizer: tree L1/L2 on Pool (GPSIMD), tail levels on DVE,
  reciprocal + scale (DVE), bn2+relu (ACT), wc conv (PE) -> plain evac;
  the final +bc bias and +x identity residual are added on the host during
  the gather (saves the DVE residual pass).

Channel permutation for x3/aggregation (s-split): partition
p <-> channel 8*(p//4)+4t+(p%4), so one 4x-replicated exp tensor serves
both 128-channel tiles.
"""

import numpy as np
import ml_dtypes

bf16_np = ml_dtypes.bfloat16

K = 7
PAD = 3
EPS = 1e-5
B, C, H, W = 2, 256, 56, 56
RB = 7               # rows per core
NQ = RB * W          # 392 per batch
ROWS = RB + 2 * PAD  # 13
WP = W + 2 * PAD     # 62
K2 = K * K
CHUNKS = [(0, 1), (1, 2), (3, 2), (5, 2)]  # (row0, nrows)
P1 = 1024
PIECE = 1024

_BUILD_CACHE = {}


def _perm_channels():
    perm = np.zeros(256, np.int64)
    for t in range(2):
        for p in range(128):
            perm[t * 128 + p] = 8 * (p // 4) + 4 * t + (p % 4)
    return perm


def _build_program():
    if "nc" in _BUILD_CACHE:
        return _BUILD_CACHE["nc"]
    import concourse.bass as bass
    import concourse.bacc as bacc
    import concourse.tile as tile
    import concourse.mybir as mybir
    from contextlib import ExitStack

    f32 = mybir.dt.float32
    f16 = mybir.dt.float16
    bf16 = mybir.dt.bfloat16
    Alu = mybir.AluOpType
    Act = mybir.ActivationFunctionType

    nc = bacc.Bacc("TRN2", target_bir_lowering=False, num_devices=8)

    xp_d = nc.dram_tensor("xp", [128, 4, ROWS * WP], f32, kind="ExternalInput")
    rsubp_d = nc.dram_tensor("rsubp", [2, K2 * NQ], f16, kind="ExternalInput")
    # packed weights: w1(2x64) | w2(2x64) | w3(2kt,2ot,128) | wc(...) |
    # cw1blk(128) | cw2(128)
    wpk_d = nc.dram_tensor("wpk", [128, 1280], bf16, kind="ExternalInput")
    cwf_d = nc.dram_tensor("cwf", [128, 256], f16, kind="ExternalInput")
    cw1pos_d = nc.dram_tensor("cw1pos", [2, 128], f16, kind="ExternalInput")
    scal_d = nc.dram_tensor("scal", [128, 13], f32, kind="ExternalInput")
    y_d = nc.dram_tensor("y", [4, 128, NQ], f32, kind="ExternalOutput")

    with tile.TileContext(nc) as tc, ExitStack() as stack:
        consts = stack.enter_context(tc.tile_pool(name="consts", bufs=1))
        xpp = stack.enter_context(tc.tile_pool(name="xpp", bufs=1))
        headsb = stack.enter_context(tc.tile_pool(name="headsb", bufs=1))
        featp = stack.enter_context(tc.tile_pool(name="featp", bufs=3))
        h2p = stack.enter_context(tc.tile_pool(name="h2p", bufs=3))
        e4p = stack.enter_context(tc.tile_pool(name="e4p", bufs=4))
        prodp = stack.enter_context(tc.tile_pool(name="prodp", bufs=3))
        zscp = stack.enter_context(tc.tile_pool(name="zscp", bufs=1))
        stripep = stack.enter_context(tc.tile_pool(name="stripep", bufs=2))
        smallp = stack.enter_context(tc.tile_pool(name="smallp", bufs=2))
        ps1p = stack.enter_context(tc.tile_pool(name="ps1p", bufs=1, space="PSUM"))
        ps2p = stack.enter_context(tc.tile_pool(name="ps2p", bufs=2, space="PSUM"))
        psxp = stack.enter_context(tc.tile_pool(name="psxp", bufs=1, space="PSUM"))

        scals = consts.tile([128, 13], f32, tag="scals")
        nc.sync.dma_start(out=scals[:], in_=scal_d[:])
        wpk = consts.tile([128, 1280], bf16, tag="wpk")
        cwf = consts.tile([128, 256], f16, tag="cwf")
        cwps = consts.tile([2, 128], f16, tag="cwps")
        w1s = wpk[:].rearrange("p (a b) -> p a b", b=64)[:, 0:2, :]
        w2s = wpk[:].rearrange("p (a b) -> p a b", b=64)[:, 2:4, :]
        w3s = wpk[:, 256:768].rearrange("p (kt ot m) -> p kt ot m",
                                        kt=2, ot=2)
        wcs = wpk[:, 768:1280].rearrange("p (kt ot m) -> p kt ot m",
                                         kt=2, ot=2)
        cw1s = cwf[:, 0:128]
        cw2s = cwf[:, 128:256]

        a1 = [scals[:, 0:1], scals[:, 1:2]]
        b1f = [scals[:, 2:3], scals[:, 3:4]]
        b1pd = scals[:, 4:5]
        b2pd = scals[:, 5:6]
        b2fd = scals[:, 6:7]
        a3p = [scals[:, 7:8], scals[:, 8:9]]
        b3fp = [scals[:, 9:10], scals[:, 10:11]]
        bcb = [scals[:, 11:12], scals[:, 12:13]]

        # input slab: slot (b*2+ct) of [128ch, ROWS*WP]; per-slot DMAs so the
        # first obn can start before the whole slab lands
        xps = xpp.tile([128, 4, ROWS * WP], f32, tag="xps")
        obn = headsb.tile([128, 4, ROWS * WP], bf16, tag="obn")
        for b in range(2):
            for ct in range(2):
                s = b * 2 + ct
                nc.sync.dma_start(out=xps[:, s, :], in_=xp_d[:, s, :])
                nc.scalar.activation(
                    out=obn[:, s, :], in_=xps[:, s, :],
                    func=Act.Relu, bias=b1f[ct], scale=a1[ct])
        nc.sync.dma_start(out=wpk[:], in_=wpk_d[:])
        nc.sync.dma_start(out=cwf[:], in_=cwf_d[:])
        nc.sync.dma_start(out=cwps[:], in_=cw1pos_d[:])

        def mm(out_ap, w_ap, rhs_ap, n, start, stop):
            # PE matmul: moving dim must be <= 512 per instruction
            for s in range(0, n, 512):
                sn = min(512, n - s)
                nc.tensor.matmul(out_ap[:, s:s + sn], w_ap,
                                 rhs_ap[:, s:s + sn], start=start, stop=stop)

        # x1s: [128 = 2b x 64rel, RB*W] f16 (center rows/cols only)
        x1s = headsb.tile([128, RB, W], f16, tag="x1s")
        psx1 = psxp.tile([128, PIECE], f32, tag="psx")
        for b in range(2):
            for kt in range(2):
                rhs = obn[:, b * 2 + kt, :].rearrange(
                    "p (r w) -> p r w", w=WP)[:, PAD:PAD + RB, PAD:PAD + W]
                nc.tensor.matmul(
                    psx1[64 * b:64 * b + 64, :NQ], w1s[:, kt, :], rhs,
                    start=(kt == 0), stop=(kt == 1))
        x1f = x1s[:].rearrange("p r w -> p (r w)")
        nc.scalar.activation(out=x1f[:, 0:W], in_=psx1[:, :W],
                             func=Act.Identity, bias=b1pd, scale=1.0)
        nc.scalar.activation(out=x1f[:, W:NQ], in_=psx1[:, W:NQ],
                             func=Act.Identity, bias=b1pd, scale=1.0)

        # x2s: [128 = 2b x 64rel, ROWS*WP] f16
        x2s = headsb.tile([128, ROWS * WP], f16, tag="x2s")
        psx2 = psxp.tile([128, PIECE], f32, tag="psx")
        for b in range(2):
            for kt in range(2):
                mm(psx2[64 * b:64 * b + 64, :], w2s[:, kt, :],
                   obn[:, b * 2 + kt, :], ROWS * WP, kt == 0, kt == 1)
        # evac rows 0..6 first so chunk 0's subtract can start early
        nc.scalar.activation(out=x2s[:, 0:7 * WP], in_=psx2[:, :7 * WP],
                             func=Act.Identity, bias=b2pd, scale=1.0)
        nc.scalar.activation(out=x2s[:, 7 * WP:], in_=psx2[:, 7 * WP:ROWS * WP],
                             func=Act.Identity, bias=b2pd, scale=1.0)

        # x3ps: slot (b*2+ot): [128 perm-ch, ROWS*WP] f16 (emitted after
        # phase1(0) so its evacs don't block the first h2/exp stream)
        x3ps = headsb.tile([128, 4, ROWS * WP], f16, tag="x3ps")

        def x3convs():
            for b in range(2):
                for ot in range(2):
                    ps3 = ps2p.tile([128, PIECE], f32, tag="ps2")
                    for kt in range(2):
                        mm(ps3, w3s[:, kt, ot, :],
                           obn[:, b * 2 + kt, :], ROWS * WP, kt == 0, kt == 1)
                    nc.scalar.activation(out=x3ps[:, b * 2 + ot, :],
                                         in_=ps3[:, :ROWS * WP], func=Act.Copy)

        chunk_state = {}
        # chunk-major offsets into rsubp_d's flat [K2*NQ]
        roffs = []
        acc = 0
        for (r0c, nr) in CHUNKS:
            roffs.append(acc)
            acc += K2 * nr * W

        def phase1(ci):
            (r0c, nr) = CHUNKS[ci]
            nqc = nr * W
            vc = K2 * nqc
            feat = featp.tile([128, vc], f16, tag="feat", name=f"feat{ci}")
            x1v = x1s[:, r0c:r0c + nr, :]
            # feat[k=(di,dj), q=(r,c)] = x1 - x2window ; one DVE op per di
            for di in range(K):
                x2w = bass.AP(
                    tensor=x2s.tensor, offset=x2s[:].offset + (r0c + di) * WP,
                    ap=[x2s[:].ap[0], [1, K], [WP, nr], [1, W]])
                x1w = bass.AP(
                    tensor=x1v.tensor, offset=x1v.offset,
                    ap=[x1v.ap[0], [0, K], x1v.ap[1], x1v.ap[2]])
                outw = bass.AP(
                    tensor=feat.tensor, offset=feat[:].offset + di * K * nqc,
                    ap=[feat[:].ap[0], [nqc, K], [W, nr], [1, W]])
                seng = nc.gpsimd if di >= 5 else nc.vector
                seng.tensor_tensor(out=outw, in0=x1w, in1=x2w,
                                   op=Alu.subtract)
            nc.vector.tensor_scalar_max(out=feat[:], in0=feat[:],
                                        scalar1=0.0)

            h2 = h2p.tile([128, vc], f16, tag="h2", name=f"h2{ci}")
            for j0 in range(0, vc, P1):
                n = min(P1, vc - j0)
                stripe = stripep.tile([2, P1], f16, tag="stripe")
                nc.sync.dma_start(
                    out=stripe[:, :n],
                    in_=rsubp_d[:, roffs[ci] + j0:roffs[ci] + j0 + n])
                ps1 = ps1p.tile([128, P1], f32, tag="ps1")
                mm(ps1, cw1s[:], feat[:, j0:j0 + n], n, True, False)
                mm(ps1, cwps[:], stripe[:, :n], n, False, True)
                nc.scalar.activation(out=h2[:, j0:j0 + n], in_=ps1[:, :n],
                                     func=Act.Relu, bias=b2fd, scale=1.0)

            e4s = []
            for b in range(2):
                e4 = e4p.tile([128, vc], f16, tag="e4", name=f"e4_{ci}_{b}")
                e4s.append(e4)
                for j0 in range(0, vc, PIECE):
                    n = min(PIECE, vc - j0)
                    ps2 = ps2p.tile([128, PIECE], f32, tag="ps2")
                    mm(ps2, cw2s[64 * b:64 * b + 64, :],
                       h2[64 * b:64 * b + 64, j0:j0 + n], n, True, True)
                    nc.scalar.activation(out=e4[:, j0:j0 + n], in_=ps2[:, :n],
                                         func=Act.Exp)
            chunk_state[ci] = e4s

        def ksum_tree(eng, t, nqc):
            # in-place pairwise sum of 49 tap-planes [128, 49, nqc] -> [:,0,:]
            def v(k, n):
                return bass.AP(tensor=t.tensor, offset=t[:].offset + k * nqc,
                               ap=[t[:].ap[0], [nqc, n], [1, nqc]])
            for (a, b_, n) in [(0, 24, 24), (0, 12, 12), (0, 6, 6), (0, 3, 3)]:
                eng.tensor_tensor(out=v(a, n), in0=v(a, n), in1=v(b_, n),
                                  op=Alu.add)
            for b_ in (1, 2, 48):
                eng.tensor_tensor(out=v(0, 1), in0=v(0, 1), in1=v(b_, 1),
                                  op=Alu.add)

        def phase2(ci, b):
            (r0c, nr) = CHUNKS[ci]
            nqc = nr * W
            e4 = chunk_state[ci][b]

            # softmax normalizer: L1+L2 on Pool, small levels on DVE
            zsc = zscp.tile([128, 24, nqc], f16, tag="zsc", name=f"zsc{ci}{b}")
            def ev(k, n):
                return bass.AP(tensor=e4.tensor, offset=e4[:].offset + k * nqc,
                               ap=[e4[:].ap[0], [nqc, n], [1, nqc]])
            zeng = (nc.vector if (ci == len(CHUNKS) - 1 and b == 1)
                    else nc.gpsimd)
            zeng.tensor_tensor(out=zsc[:, 0:24, :], in0=ev(0, 24),
                               in1=ev(24, 24), op=Alu.add)
            nc.vector.tensor_tensor(out=zsc[:, 0:12, :], in0=zsc[:, 0:12, :],
                                    in1=zsc[:, 12:24, :], op=Alu.add)
            for (a, b_, n) in [(0, 6, 6), (0, 3, 3)]:
                nc.vector.tensor_tensor(out=zsc[:, a:a + n, :],
                                        in0=zsc[:, a:a + n, :],
                                        in1=zsc[:, b_:b_ + n, :], op=Alu.add)
            for b_ in (1, 2):
                nc.vector.tensor_tensor(out=zsc[:, 0, :], in0=zsc[:, 0, :],
                                        in1=zsc[:, b_, :], op=Alu.add)
            nc.vector.tensor_tensor(out=zsc[:, 0, :], in0=zsc[:, 0, :],
                                    in1=ev(48, 1), op=Alu.add)

            zf = smallp.tile([128, nqc], f32, tag="zf")
            rz = smallp.tile([128, nqc], f32, tag="rz")
            nc.vector.tensor_copy(out=zf[:], in_=zsc[:, 0, :])
            nc.vector.reciprocal(out=rz[:], in_=zf[:])

            outb = []
            for t in range(2):
                prods = prodp.tile([128, K2, nqc], f16, tag="prods",
                                   name=f"prods{ci}{b}{t}")
                x3v = x3ps[:, b * 2 + t, :]
                for di in range(K):
                    x3w = bass.AP(
                        tensor=x3v.tensor,
                        offset=x3v.offset + (r0c + di) * WP,
                        ap=[x3v.ap[0], [1, K], [WP, nr], [1, W]])
                    e4w = bass.AP(
                        tensor=e4.tensor, offset=e4[:].offset + di * K * nqc,
                        ap=[e4[:].ap[0], [nqc, K], [W, nr], [1, W]])
                    outw = bass.AP(
                        tensor=prods.tensor,
                        offset=prods[:].offset + di * K * nqc,
                        ap=[prods[:].ap[0], [nqc, K], [W, nr], [1, W]])
                    # di=0 group on Pool: fills its idle windows; the tree
                    # (DVE) consumes all 49 taps regardless of the writer
                    eng = nc.gpsimd if di <= 1 else nc.vector
                    eng.tensor_tensor(out=outw, in0=e4w, in1=x3w,
                                      op=Alu.mult)
                ksum_tree(nc.vector, prods, nqc)

                ob = smallp.tile([128, nqc], f32, tag=f"ob{t}", name=f"ob{t}")
                ob2 = smallp.tile([128, nqc], bf16, tag=f"ob2{t}",
                                  name=f"ob2_{ci}{b}{t}")
                outb.append(ob2)
                nc.vector.tensor_tensor(out=ob[:], in0=prods[:, 0, :],
                                        in1=rz[:], op=Alu.mult)
                nc.scalar.activation(out=ob2[:], in_=ob[:], func=Act.Relu,
                                     bias=b3fp[t], scale=a3p[t])

            for oo in range(2):
                psw = psxp.tile([128, PIECE], f32, tag="psx")
                for kt in range(2):
                    mm(psw, wcs[:, kt, oo, :], outb[kt][:], nqc,
                       kt == 0, kt == 1)
                ysb = smallp.tile([128, nqc], f32, tag=f"ysb{oo}",
                                  name=f"ysb{ci}{b}{oo}")
                nc.scalar.activation(out=ysb[:], in_=psw[:, :nqc],
                                     func=Act.Copy)
                nc.sync.dma_start(
                    out=y_d[b * 2 + oo][:, r0c * W:(r0c + nr) * W],
                    in_=ysb[:])

        phase1(0)
        x3convs()
        phase1(1)
        phase2(0, 0)
        phase2(0, 1)
        phase1(2)
        phase2(1, 0)
        phase2(1, 1)
        phase1(3)
        phase2(2, 0)
        phase2(2, 1)
        phase2(3, 0)
        phase2(3, 1)

    nc.compile()
    _BUILD_CACHE["nc"] = nc
    return nc


def _host_prep(inputs):
    f = {k: np.asarray(v, np.float32) for k, v in inputs.items()}

    def fold(n):
        a = f[n + "_g"] / np.sqrt(f[n + "_rv"] + EPS)
        return a, f[n + "_b"] - f[n + "_rm"] * a

    a1, b1f = fold("bn1")
    ac, bc1 = fold("cwbn1")
    a2, b2f = fold("cwbn2")
    a3, b3f = fold("bn2")

    W1p = ac[:64, None] * f["w1"]
    b1p = ac[:64] * f["b1"] + bc1[:64]
    W2p = ac[:64, None] * f["w2"]
    b2p = ac[:64] * f["b2"]
    cw1p = a2[:, None] * f["cw1"]

    perm = _perm_channels()
    w3p = f["w3"][perm]
    a3p = a3[perm]
    b3fp = b3f[perm]
    rep = np.arange(128) // 4
    cw2r = f["cw2"][rep]

    # position encoding: relu(bn(subp)) on host
    locw = np.tile(np.linspace(-1.0, 1.0, W, dtype=np.float32)[None, :], (H, 1))
    loch = np.tile(np.linspace(-1.0, 1.0, H, dtype=np.float32)[:, None], (1, W))
    loc = np.stack([locw, loch], 0)
    p = np.einsum("chw,oc->ohw", loc, f["pw"]) + f["pb"][:, None, None]
    pp = np.pad(p, ((0, 0), (PAD, PAD), (PAD, PAD)), mode="reflect")
    pu = np.stack([pp[:, i:i + H, j:j + W] for i in range(K) for j in range(K)], 1)
    subp = p[:, None] - pu
    rsubp = np.maximum(ac[64:66, None, None, None] * subp
                       + bc1[64:66, None, None, None], 0).astype(np.float16)

    xpad = np.pad(f["x"], ((0, 0), (0, 0), (PAD, PAD), (PAD, PAD)),
                  mode="reflect")

    w1T = np.ascontiguousarray(W1p.T).reshape(2, 128, 64)
    w2T = np.ascontiguousarray(W2p.T).reshape(2, 128, 64)
    w3T = np.empty((2, 128, 2, 128), np.float32)
    wcT = np.empty((2, 128, 2, 128), np.float32)
    wc_perm = f["wc"][:, perm]
    for kt in range(2):
        for ot in range(2):
            w3T[kt, :, ot, :] = w3p[ot * 128:(ot + 1) * 128,
                                    kt * 128:(kt + 1) * 128].T
            wcT[kt, :, ot, :] = wc_perm[ot * 128:(ot + 1) * 128,
                                        kt * 128:(kt + 1) * 128].T

    cw1blk = np.zeros((128, 128), np.float32)
    cw1blk[0:64, 0:64] = cw1p[:, :64].T
    cw1blk[64:128, 64:128] = cw1p[:, :64].T
    cw1pos = np.zeros((2, 128), np.float32)
    cw1pos[:, 0:64] = cw1p[:, 64:66].T
    cw1pos[:, 64:128] = cw1p[:, 64:66].T
    cw2T = np.ascontiguousarray(np.concatenate([cw2r.T, cw2r.T], axis=0))

    scal = np.zeros((128, 13), np.float32)
    scal[:, 0] = a1[:128]; scal[:, 1] = a1[128:]
    scal[:, 2] = b1f[:128]; scal[:, 3] = b1f[128:]
    scal[:64, 4] = b1p; scal[64:, 4] = b1p
    scal[:64, 5] = b2p; scal[64:, 5] = b2p
    scal[:64, 6] = b2f; scal[64:, 6] = b2f
    scal[:, 7] = a3p[:128]; scal[:, 8] = a3p[128:]
    scal[:, 9] = b3fp[:128]; scal[:, 10] = b3fp[128:]
    scal[:, 11] = f["bc"][:128]; scal[:, 12] = f["bc"][128:]

    wpk = np.zeros((128, 1280), np.float32)
    wpk[:, 0:64] = w1T[0]; wpk[:, 64:128] = w1T[1]
    wpk[:, 128:192] = w2T[0]; wpk[:, 192:256] = w2T[1]
    wpk[:, 256:768] = w3T.reshape(2, 128, 256).transpose(1, 0, 2).reshape(
        128, 512)
    wpk[:, 768:1280] = wcT.reshape(2, 128, 256).transpose(1, 0, 2).reshape(
        128, 512)
    cwf = np.concatenate([cw1blk, cw2T], axis=1)
    shared = dict(wpk=wpk.astype(bf16_np), cwf=cwf.astype(np.float16),
                  cw1pos=cw1pos.astype(np.float16), scal=scal)

    in_maps = []
    for core in range(8):
        r0 = RB * core
        m = dict(shared)
        # xp: [128ch, 4 = (b*2+ct), ROWS*WP]
        slab = xpad[:, :, r0:r0 + ROWS, :]          # [2, 256, 13, 62]
        xp = np.empty((128, 4, ROWS * WP), np.float32)
        for b in range(2):
            for ct in range(2):
                xp[:, b * 2 + ct, :] = slab[b, ct * 128:(ct + 1) * 128].reshape(
                    128, ROWS * WP)
        m["xp"] = xp
        # rsubp: chunk-major flat [2, K2*NQ]
        rs = rsubp[:, :, r0:r0 + RB, :]             # [2, 49, 7, 56]
        parts = []
        for (r0c, nr) in CHUNKS:
            parts.append(rs[:, :, r0c:r0c + nr, :].reshape(2, -1))
        m["rsubp"] = np.ascontiguousarray(np.concatenate(parts, axis=1))
        in_maps.append(m)
    return in_maps


def kernel(**inputs):
    from concourse.bass_utils import run_bass_kernel_spmd
    nc = _build_program()
    in_maps = _host_prep(inputs)
    res = run_bass_kernel_spmd(nc, in_maps, core_ids=list(range(8)))
    global LAST_RESULTS
    LAST_RESULTS = res
    y = np.zeros((B, C, H, W), np.float32)
    for core in range(8):
        r0 = RB * core
        yc = res.results[core]["y"]                 # [4, 128, NQ]
        for b in range(2):
            for ot in range(2):
                y[b, ot * 128:(ot + 1) * 128, r0:r0 + RB, :] = (
                    yc[b * 2 + ot].reshape(128, RB, W))
    y += np.asarray(inputs["bc"], np.float32).reshape(1, C, 1, 1)
    y += np.asarray(inputs["x"], np.float32)
    return y


# revision 51
# speedup vs baseline: 1.0833x; 1.0261x over previous
"""SAN Bottleneck (pairwise self-attention) Trainium2 kernel, v2.

Sharding: 8 cores x 7 output rows (H=56), each core handles BOTH batches.
The rel=64-channel tensors (x1/x2/feat/h2) pack the two batches across the
128 partitions, halving the free-dim size of the windowed subtract, relu,
mm1 (one K=128 block-diagonal matmul computes both batches per column) and
the h2 relu-evacuation.

Per-core pipeline (batchnorms folded into per-channel scale/bias on host;
4 row-chunks, piece-wise PSUM streaming for mm1/mm2):
  bn1+relu (ACT) -> x1/x2/x3 1x1 convs (bf16 matmuls, batch-stacked M=128)
  feat = x1 - window(x2)  (DVE fp16 2x, one op per di)  -> relu (DVE 4x)
  mm1: block-diag cw1 (K=128) + position pass (K=2, rsubp streamed from HBM)
  relu evac (ACT) -> mm2 per batch (K=64, M=128 4x-replicated heads)
  exp evac (ACT, no bias: softmax is shift-invariant per (head,q))
  aggregation: windowed products + pairwise 49-tap tree (DVE fp16 2x)
  softmax normalizer: tree L1/L2 on Pool (GPSIMD), tail levels on DVE,
  reciprocal + scale (DVE), bn2+relu (ACT), wc conv (PE) -> plain evac;
  the final +bc bias and +x identity residual are added on the host during
  the gather (saves the DVE residual pass).

Channel permutation for x3/aggregation (s-split): partition
p <-> channel 8*(p//4)+4t+(p%4), so one 4x-replicated exp tensor serves
both 128-channel tiles.
"""

import numpy as np
import ml_dtypes

bf16_np = ml_dtypes.bfloat16

K = 7
PAD = 3
EPS = 1e-5
B, C, H, W = 2, 256, 56, 56
RB = 7               # rows per core
NQ = RB * W          # 392 per batch
ROWS = RB + 2 * PAD  # 13
WP = W + 2 * PAD     # 62
K2 = K * K
CHUNKS = [(0, 1), (1, 2), (3, 2), (5, 2)]  # (row0, nrows)
P1 = 1024
PIECE = 1024

_BUILD_CACHE = {}


def _perm_channels():
    perm = np.zeros(256, np.int64)
    for t in range(2):
        for p in range(128):
            perm[t * 128 + p] = 8 * (p // 4) + 4 * t + (p % 4)
    return perm


def _build_program():
    if "nc" in _BUILD_CACHE:
        return _BUILD_CACHE["nc"]
    import concourse.bass as bass
    import concourse.bacc as bacc
    import concourse.tile as tile
    import concourse.mybir as mybir
    from contextlib import ExitStack

    f32 = mybir.dt.float32
    f16 = mybir.dt.float16
    bf16 = mybir.dt.bfloat16
    Alu = mybir.AluOpType
    Act = mybir.ActivationFunctionType

    nc = bacc.Bacc("TRN2", target_bir_lowering=False, num_devices=8)

    xp_d = nc.dram_tensor("xp", [128, 4, ROWS * WP], bf16, kind="ExternalInput")
    rsubp_d = nc.dram_tensor("rsubp", [2, K2 * NQ], f16, kind="ExternalInput")
    # packed weights: w1(2x64) | w2(2x64) | w3(2kt,2ot,128) | wc(...) |
    # cw1blk(128) | cw2(128)
    wpk_d = nc.dram_tensor("wpk", [128, 1280], bf16, kind="ExternalInput")
    cwf_d = nc.dram_tensor("cwf", [128, 256], f16, kind="ExternalInput")
    cw1pos_d = nc.dram_tensor("cw1pos", [2, 128], f16, kind="ExternalInput")
    scal_d = nc.dram_tensor("scal", [128, 13], f32, kind="ExternalInput")
    y_d = nc.dram_tensor("y", [4, 128, NQ], f32, kind="ExternalOutput")

    with tile.TileContext(nc) as tc, ExitStack() as stack:
        consts = stack.enter_context(tc.tile_pool(name="consts", bufs=1))
        xpp = stack.enter_context(tc.tile_pool(name="xpp", bufs=1))
        headsb = stack.enter_context(tc.tile_pool(name="headsb", bufs=1))
        featp = stack.enter_context(tc.tile_pool(name="featp", bufs=3))
        h2p = stack.enter_context(tc.tile_pool(name="h2p", bufs=3))
        e4p = stack.enter_context(tc.tile_pool(name="e4p", bufs=4))
        prodp = stack.enter_context(tc.tile_pool(name="prodp", bufs=3))
        zscp = stack.enter_context(tc.tile_pool(name="zscp", bufs=1))
        stripep = stack.enter_context(tc.tile_pool(name="stripep", bufs=2))
        smallp = stack.enter_context(tc.tile_pool(name="smallp", bufs=2))
        ps1p = stack.enter_context(tc.tile_pool(name="ps1p", bufs=1, space="PSUM"))
        ps2p = stack.enter_context(tc.tile_pool(name="ps2p", bufs=2, space="PSUM"))
        psxp = stack.enter_context(tc.tile_pool(name="psxp", bufs=1, space="PSUM"))

        scals = consts.tile([128, 13], f32, tag="scals")
        nc.sync.dma_start(out=scals[:], in_=scal_d[:])
        wpk = consts.tile([128, 1280], bf16, tag="wpk")
        cwf = consts.tile([128, 256], f16, tag="cwf")
        cwps = consts.tile([2, 128], f16, tag="cwps")
        w1s = wpk[:].rearrange("p (a b) -> p a b", b=64)[:, 0:2, :]
        w2s = wpk[:].rearrange("p (a b) -> p a b", b=64)[:, 2:4, :]
        w3s = wpk[:, 256:768].rearrange("p (kt ot m) -> p kt ot m",
                                        kt=2, ot=2)
        wcs = wpk[:, 768:1280].rearrange("p (kt ot m) -> p kt ot m",
                                         kt=2, ot=2)
        cw1s = cwf[:, 0:128]
        cw2s = cwf[:, 128:256]

        a1 = [scals[:, 0:1], scals[:, 1:2]]
        b1f = [scals[:, 2:3], scals[:, 3:4]]
        b1pd = scals[:, 4:5]
        b2pd = scals[:, 5:6]
        b2fd = scals[:, 6:7]
        a3p = [scals[:, 7:8], scals[:, 8:9]]
        b3fp = [scals[:, 9:10], scals[:, 10:11]]
        bcb = [scals[:, 11:12], scals[:, 12:13]]

        # input slab: slot (b*2+ct) of [128ch, ROWS*WP]; per-slot DMAs so the
        # first obn can start before the whole slab lands
        xps = xpp.tile([128, 4, ROWS * WP], bf16, tag="xps")
        obn = headsb.tile([128, 4, ROWS * WP], bf16, tag="obn")
        for b in range(2):
            for ct in range(2):
                s = b * 2 + ct
                nc.sync.dma_start(out=xps[:, s, :], in_=xp_d[:, s, :])
                nc.scalar.activation(
                    out=obn[:, s, :], in_=xps[:, s, :],
                    func=Act.Relu, bias=b1f[ct], scale=a1[ct])
        nc.sync.dma_start(out=wpk[:], in_=wpk_d[:])
        nc.sync.dma_start(out=cwf[:], in_=cwf_d[:])
        nc.sync.dma_start(out=cwps[:], in_=cw1pos_d[:])

        def mm(out_ap, w_ap, rhs_ap, n, start, stop):
            # PE matmul: moving dim must be <= 512 per instruction
            for s in range(0, n, 512):
                sn = min(512, n - s)
                nc.tensor.matmul(out_ap[:, s:s + sn], w_ap,
                                 rhs_ap[:, s:s + sn], start=start, stop=stop)

        # x1s: [128 = 2b x 64rel, RB*W] f16 (center rows/cols only)
        x1s = headsb.tile([128, RB, W], f16, tag="x1s")
        psx1 = psxp.tile([128, PIECE], f32, tag="psx")
        for b in range(2):
            for kt in range(2):
                rhs = obn[:, b * 2 + kt, :].rearrange(
                    "p (r w) -> p r w", w=WP)[:, PAD:PAD + RB, PAD:PAD + W]
                nc.tensor.matmul(
                    psx1[64 * b:64 * b + 64, :NQ], w1s[:, kt, :], rhs,
                    start=(kt == 0), stop=(kt == 1))
        x1f = x1s[:].rearrange("p r w -> p (r w)")
        nc.scalar.activation(out=x1f[:, 0:W], in_=psx1[:, :W],
                             func=Act.Identity, bias=b1pd, scale=1.0)
        nc.scalar.activation(out=x1f[:, W:NQ], in_=psx1[:, W:NQ],
                             func=Act.Identity, bias=b1pd, scale=1.0)

        # x2s: [128 = 2b x 64rel, ROWS*WP] f16
        x2s = headsb.tile([128, ROWS * WP], f16, tag="x2s")
        psx2 = psxp.tile([128, PIECE], f32, tag="psx")
        for b in range(2):
            for kt in range(2):
                mm(psx2[64 * b:64 * b + 64, :], w2s[:, kt, :],
                   obn[:, b * 2 + kt, :], ROWS * WP, kt == 0, kt == 1)
        # evac rows 0..6 first so chunk 0's subtract can start early
        nc.scalar.activation(out=x2s[:, 0:7 * WP], in_=psx2[:, :7 * WP],
                             func=Act.Identity, bias=b2pd, scale=1.0)
        nc.scalar.activation(out=x2s[:, 7 * WP:], in_=psx2[:, 7 * WP:ROWS * WP],
                             func=Act.Identity, bias=b2pd, scale=1.0)

        # x3ps: slot (b*2+ot): [128 perm-ch, ROWS*WP] f16 (emitted after
        # phase1(0) so its evacs don't block the first h2/exp stream)
        x3ps = headsb.tile([128, 4, ROWS * WP], f16, tag="x3ps")

        def x3convs():
            for b in range(2):
                for ot in range(2):
                    ps3 = ps2p.tile([128, PIECE], f32, tag="ps2")
                    for kt in range(2):
                        mm(ps3, w3s[:, kt, ot, :],
                           obn[:, b * 2 + kt, :], ROWS * WP, kt == 0, kt == 1)
                    nc.scalar.activation(out=x3ps[:, b * 2 + ot, :],
                                         in_=ps3[:, :ROWS * WP], func=Act.Copy)

        chunk_state = {}
        # chunk-major offsets into rsubp_d's flat [K2*NQ]
        roffs = []
        acc = 0
        for (r0c, nr) in CHUNKS:
            roffs.append(acc)
            acc += K2 * nr * W

        def phase1(ci):
            (r0c, nr) = CHUNKS[ci]
            nqc = nr * W
            vc = K2 * nqc
            feat = featp.tile([128, vc], f16, tag="feat", name=f"feat{ci}")
            x1v = x1s[:, r0c:r0c + nr, :]
            # feat[k=(di,dj), q=(r,c)] = x1 - x2window ; one DVE op per di
            for di in range(K):
                x2w = bass.AP(
                    tensor=x2s.tensor, offset=x2s[:].offset + (r0c + di) * WP,
                    ap=[x2s[:].ap[0], [1, K], [WP, nr], [1, W]])
                x1w = bass.AP(
                    tensor=x1v.tensor, offset=x1v.offset,
                    ap=[x1v.ap[0], [0, K], x1v.ap[1], x1v.ap[2]])
                outw = bass.AP(
                    tensor=feat.tensor, offset=feat[:].offset + di * K * nqc,
                    ap=[feat[:].ap[0], [nqc, K], [W, nr], [1, W]])
                seng = nc.gpsimd if di >= 5 else nc.vector
                seng.tensor_tensor(out=outw, in0=x1w, in1=x2w,
                                   op=Alu.subtract)
            nc.vector.tensor_scalar_max(out=feat[:], in0=feat[:],
                                        scalar1=0.0)

            h2 = h2p.tile([128, vc], f16, tag="h2", name=f"h2{ci}")
            for j0 in range(0, vc, P1):
                n = min(P1, vc - j0)
                stripe = stripep.tile([2, P1], f16, tag="stripe")
                nc.sync.dma_start(
                    out=stripe[:, :n],
                    in_=rsubp_d[:, roffs[ci] + j0:roffs[ci] + j0 + n])
                ps1 = ps1p.tile([128, P1], f32, tag="ps1")
                mm(ps1, cw1s[:], feat[:, j0:j0 + n], n, True, False)
                mm(ps1, cwps[:], stripe[:, :n], n, False, True)
                nc.scalar.activation(out=h2[:, j0:j0 + n], in_=ps1[:, :n],
                                     func=Act.Relu, bias=b2fd, scale=1.0)

            e4s = []
            for b in range(2):
                e4 = e4p.tile([128, vc], f16, tag="e4", name=f"e4_{ci}_{b}")
                e4s.append(e4)
                for j0 in range(0, vc, PIECE):
                    n = min(PIECE, vc - j0)
                    ps2 = ps2p.tile([128, PIECE], f32, tag="ps2")
                    mm(ps2, cw2s[64 * b:64 * b + 64, :],
                       h2[64 * b:64 * b + 64, j0:j0 + n], n, True, True)
                    nc.scalar.activation(out=e4[:, j0:j0 + n], in_=ps2[:, :n],
                                         func=Act.Exp)
            chunk_state[ci] = e4s

        def ksum_tree(eng, t, nqc):
            # in-place pairwise sum of 49 tap-planes [128, 49, nqc] -> [:,0,:]
            def v(k, n):
                return bass.AP(tensor=t.tensor, offset=t[:].offset + k * nqc,
                               ap=[t[:].ap[0], [nqc, n], [1, nqc]])
            for (a, b_, n) in [(0, 24, 24), (0, 12, 12), (0, 6, 6), (0, 3, 3)]:
                eng.tensor_tensor(out=v(a, n), in0=v(a, n), in1=v(b_, n),
                                  op=Alu.add)
            for b_ in (1, 2, 48):
                eng.tensor_tensor(out=v(0, 1), in0=v(0, 1), in1=v(b_, 1),
                                  op=Alu.add)

        def phase2(ci, b):
            (r0c, nr) = CHUNKS[ci]
            nqc = nr * W
            e4 = chunk_state[ci][b]

            # softmax normalizer: L1+L2 on Pool, small levels on DVE
            zsc = zscp.tile([128, 24, nqc], f16, tag="zsc", name=f"zsc{ci}{b}")
            def ev(k, n):
                return bass.AP(tensor=e4.tensor, offset=e4[:].offset + k * nqc,
                               ap=[e4[:].ap[0], [nqc, n], [1, nqc]])
            zeng = (nc.vector if (ci == len(CHUNKS) - 1 and b == 1)
                    else nc.gpsimd)
            zeng.tensor_tensor(out=zsc[:, 0:24, :], in0=ev(0, 24),
                               in1=ev(24, 24), op=Alu.add)
            nc.vector.tensor_tensor(out=zsc[:, 0:12, :], in0=zsc[:, 0:12, :],
                                    in1=zsc[:, 12:24, :], op=Alu.add)
            for (a, b_, n) in [(0, 6, 6), (0, 3, 3)]:
                nc.vector.tensor_tensor(out=zsc[:, a:a + n, :],
                                        in0=zsc[:, a:a + n, :],
                                        in1=zsc[:, b_:b_ + n, :], op=Alu.add)
            for b_ in (1, 2):
                nc.vector.tensor_tensor(out=zsc[:, 0, :], in0=zsc[:, 0, :],
                                        in1=zsc[:, b_, :], op=Alu.add)
            nc.vector.tensor_tensor(out=zsc[:, 0, :], in0=zsc[:, 0, :],
                                    in1=ev(48, 1), op=Alu.add)

            zf = smallp.tile([128, nqc], f32, tag="zf")
            rz = smallp.tile([128, nqc], f32, tag="rz")
            nc.vector.tensor_copy(out=zf[:], in_=zsc[:, 0, :])
            nc.vector.reciprocal(out=rz[:], in_=zf[:])

            outb = []
            for t in range(2):
                prods = prodp.tile([128, K2, nqc], f16, tag="prods",
                                   name=f"prods{ci}{b}{t}")
                x3v = x3ps[:, b * 2 + t, :]
                for di in range(K):
                    x3w = bass.AP(
                        tensor=x3v.tensor,
                        offset=x3v.offset + (r0c + di) * WP,
                        ap=[x3v.ap[0], [1, K], [WP, nr], [1, W]])
                    e4w = bass.AP(
                        tensor=e4.tensor, offset=e4[:].offset + di * K * nqc,
                        ap=[e4[:].ap[0], [nqc, K], [W, nr], [1, W]])
                    outw = bass.AP(
                        tensor=prods.tensor,
                        offset=prods[:].offset + di * K * nqc,
                        ap=[prods[:].ap[0], [nqc, K], [W, nr], [1, W]])
                    # di=0 group on Pool: fills its idle windows; the tree
                    # (DVE) consumes all 49 taps regardless of the writer
                    eng = nc.gpsimd if di <= 1 else nc.vector
                    eng.tensor_tensor(out=outw, in0=e4w, in1=x3w,
                                      op=Alu.mult)
                ksum_tree(nc.vector, prods, nqc)

                ob = smallp.tile([128, nqc], f32, tag=f"ob{t}", name=f"ob{t}")
                ob2 = smallp.tile([128, nqc], bf16, tag=f"ob2{t}",
                                  name=f"ob2_{ci}{b}{t}")
                outb.append(ob2)
                nc.gpsimd.tensor_tensor(out=ob[:], in0=prods[:, 0, :],
                                        in1=rz[:], op=Alu.mult)
                nc.scalar.activation(out=ob2[:], in_=ob[:], func=Act.Relu,
                                     bias=b3fp[t], scale=a3p[t])

            for oo in range(2):
                psw = psxp.tile([128, PIECE], f32, tag="psx")
                for kt in range(2):
                    mm(psw, wcs[:, kt, oo, :], outb[kt][:], nqc,
                       kt == 0, kt == 1)
                ysb = smallp.tile([128, nqc], f32, tag=f"ysb{oo}",
                                  name=f"ysb{ci}{b}{oo}")
                nc.scalar.activation(out=ysb[:], in_=psw[:, :nqc],
                                     func=Act.Copy)
                nc.sync.dma_start(
                    out=y_d[b * 2 + oo][:, r0c * W:(r0c + nr) * W],
                    in_=ysb[:])

        phase1(0)
        x3convs()
        phase1(1)
        phase2(0, 0)
        phase2(0, 1)
        phase1(2)
        phase2(1, 0)
        phase2(1, 1)
        phase1(3)
        phase2(2, 0)
        phase2(2, 1)
        phase2(3, 0)
        phase2(3, 1)

    nc.compile()
    _BUILD_CACHE["nc"] = nc
    return nc


def _host_prep(inputs):
    f = {k: np.asarray(v, np.float32) for k, v in inputs.items()}

    def fold(n):
        a = f[n + "_g"] / np.sqrt(f[n + "_rv"] + EPS)
        return a, f[n + "_b"] - f[n + "_rm"] * a

    a1, b1f = fold("bn1")
    ac, bc1 = fold("cwbn1")
    a2, b2f = fold("cwbn2")
    a3, b3f = fold("bn2")

    W1p = ac[:64, None] * f["w1"]
    b1p = ac[:64] * f["b1"] + bc1[:64]
    W2p = ac[:64, None] * f["w2"]
    b2p = ac[:64] * f["b2"]
    cw1p = a2[:, None] * f["cw1"]

    perm = _perm_channels()
    w3p = f["w3"][perm]
    a3p = a3[perm]
    b3fp = b3f[perm]
    rep = np.arange(128) // 4
    cw2r = f["cw2"][rep]

    # position encoding: relu(bn(subp)) on host
    locw = np.tile(np.linspace(-1.0, 1.0, W, dtype=np.float32)[None, :], (H, 1))
    loch = np.tile(np.linspace(-1.0, 1.0, H, dtype=np.float32)[:, None], (1, W))
    loc = np.stack([locw, loch], 0)
    p = np.einsum("chw,oc->ohw", loc, f["pw"]) + f["pb"][:, None, None]
    pp = np.pad(p, ((0, 0), (PAD, PAD), (PAD, PAD)), mode="reflect")
    pu = np.stack([pp[:, i:i + H, j:j + W] for i in range(K) for j in range(K)], 1)
    subp = p[:, None] - pu
    rsubp = np.maximum(ac[64:66, None, None, None] * subp
                       + bc1[64:66, None, None, None], 0).astype(np.float16)

    xpad = np.pad(f["x"], ((0, 0), (0, 0), (PAD, PAD), (PAD, PAD)),
                  mode="reflect")

    w1T = np.ascontiguousarray(W1p.T).reshape(2, 128, 64)
    w2T = np.ascontiguousarray(W2p.T).reshape(2, 128, 64)
    w3T = np.empty((2, 128, 2, 128), np.float32)
    wcT = np.empty((2, 128, 2, 128), np.float32)
    wc_perm = f["wc"][:, perm]
    for kt in range(2):
        for ot in range(2):
            w3T[kt, :, ot, :] = w3p[ot * 128:(ot + 1) * 128,
                                    kt * 128:(kt + 1) * 128].T
            wcT[kt, :, ot, :] = wc_perm[ot * 128:(ot + 1) * 128,
                                        kt * 128:(kt + 1) * 128].T

    cw1blk = np.zeros((128, 128), np.float32)
    cw1blk[0:64, 0:64] = cw1p[:, :64].T
    cw1blk[64:128, 64:128] = cw1p[:, :64].T
    cw1pos = np.zeros((2, 128), np.float32)
    cw1pos[:, 0:64] = cw1p[:, 64:66].T
    cw1pos[:, 64:128] = cw1p[:, 64:66].T
    cw2T = np.ascontiguousarray(np.concatenate([cw2r.T, cw2r.T], axis=0))

    scal = np.zeros((128, 13), np.float32)
    scal[:, 0] = a1[:128]; scal[:, 1] = a1[128:]
    scal[:, 2] = b1f[:128]; scal[:, 3] = b1f[128:]
    scal[:64, 4] = b1p; scal[64:, 4] = b1p
    scal[:64, 5] = b2p; scal[64:, 5] = b2p
    scal[:64, 6] = b2f; scal[64:, 6] = b2f
    scal[:, 7] = a3p[:128]; scal[:, 8] = a3p[128:]
    scal[:, 9] = b3fp[:128]; scal[:, 10] = b3fp[128:]
    scal[:, 11] = f["bc"][:128]; scal[:, 12] = f["bc"][128:]

    wpk = np.zeros((128, 1280), np.float32)
    wpk[:, 0:64] = w1T[0]; wpk[:, 64:128] = w1T[1]
    wpk[:, 128:192] = w2T[0]; wpk[:, 192:256] = w2T[1]
    wpk[:, 256:768] = w3T.reshape(2, 128, 256).transpose(1, 0, 2).reshape(
        128, 512)
    wpk[:, 768:1280] = wcT.reshape(2, 128, 256).transpose(1, 0, 2).reshape(
        128, 512)
    cwf = np.concatenate([cw1blk, cw2T], axis=1)
    shared = dict(wpk=wpk.astype(bf16_np), cwf=cwf.astype(np.float16),
                  cw1pos=cw1pos.astype(np.float16), scal=scal)

    in_maps = []
    for core in range(8):
        r0 = RB * core
        m = dict(shared)
        # xp: [128ch, 4 = (b*2+ct), ROWS*WP]
        slab = xpad[:, :, r0:r0 + ROWS, :]          # [2, 256, 13, 62]
        xp = np.empty((128, 4, ROWS * WP), bf16_np)
        for b in range(2):
            for ct in range(2):
                xp[:, b * 2 + ct, :] = slab[b, ct * 128:(ct + 1) * 128].reshape(
                    128, ROWS * WP)
        m["xp"] = xp
        # rsubp: chunk-major flat [2, K2*NQ]
        rs = rsubp[:, :, r0:r0 + RB, :]             # [2, 49, 7, 56]
        parts = []
        for (r0c, nr) in CHUNKS:
            parts.append(rs[:, :, r0c:r0c + nr, :].reshape(2, -1))
        m["rsubp"] = np.ascontiguousarray(np.concatenate(parts, axis=1))
        in_maps.append(m)
    return in_maps


def kernel(**inputs):
    from concourse.bass_utils import run_bass_kernel_spmd
    nc = _build_program()
    in_maps = _host_prep(inputs)
    res = run_bass_kernel_spmd(nc, in_maps, core_ids=list(range(8)))
    global LAST_RESULTS
    LAST_RESULTS = res
    y = np.zeros((B, C, H, W), np.float32)
    for core in range(8):
        r0 = RB * core
        yc = res.results[core]["y"]                 # [4, 128, NQ]
        for b in range(2):
            for ot in range(2):
                y[b, ot * 128:(ot + 1) * 128, r0:r0 + RB, :] = (
                    yc[b * 2 + ot].reshape(128, RB, W))
    y += np.asarray(inputs["bc"], np.float32).reshape(1, C, 1, 1)
    y += np.asarray(inputs["x"], np.float32)
    return y
